# revision 1
# baseline (speedup 1.0000x reference)
"""Cross-attention Trainium2 kernel (nn_CrossAttention_7627861918199).

Full-input contract: kernel(**inputs) takes the unsharded numpy inputs and
returns the full [B, NQ, D] float32 output.

Sharding: 8 cores = (batch b, query-half qh); core c handles batch c//2,
queries [(c%2)*512, (c%2)*512+512).  No collectives.  Each core runs a fused
attention pipeline:
  qT = Wq @ xT            (inner on partitions)
  kT = Wk @ ctxT          (kdim on partitions)
  v  = ctx @ Wv^T         (nkv on partitions, + ones column for softmax sums)
  per head: S^T chunks [128kv, nq] = k @ qT -> exp (ACT, scale=1/8)
            -> P^T bf16 -> O^T_aug[65, nq] += v_aug^T @ P^T   (PSUM accum)
  O^T = O^T_aug[:64] * recip(O^T_aug[64])   (softmax normalize)
  y = O @ Wo^T + bo
All matmul inputs bf16 (fp32 accumulate); host pre-transposes x/context/weights
so the kernel needs no on-device transposes.
"""

import os
import numpy as np
import ml_dtypes

HEADS = 8
DIM_HEAD = 64
D = 512          # QUERY_DIM == inner dim
B, NQ, NKV = 4, 1024, 4096
N_CORES = 8
NQ_SH = B * NQ // N_CORES   # 512 queries per core
P = 128
DC = D // P                  # 4 contraction chunks of 128
NCHUNK = NKV // P            # 32 kv chunks of 128
NT = NKV // 512              # 8 n-tiles for kT
BF16 = ml_dtypes.bfloat16

# exp batch: GRP score chunks share one ACT instruction / psum tile
GRP = int(os.environ.get("KERNEL_EXP_GRP", "2"))
NGRP = NCHUNK // GRP

_PROGRAMS = {}


def _build(need_mask: bool, debug_dumps: bool = False, num_devices: int = N_CORES):
    import concourse.mybir as mybir
    import concourse.tile as tile
    from concourse import bacc

    dt = mybir.dt
    f32, bf = dt.float32, dt.bfloat16
    nq = NQ_SH

    nc = bacc.Bacc("TRN2", target_bir_lowering=False, debug=False,
                   num_devices=num_devices)

    xT = nc.dram_tensor("xT", [D, nq], bf, kind="ExternalInput").ap()
    ctxT = nc.dram_tensor("ctxT", [D, NKV], bf, kind="ExternalInput").ap()
    wqT = nc.dram_tensor("wqT", [D, D], bf, kind="ExternalInput").ap()
    wkT = nc.dram_tensor("wkT", [D, D], bf, kind="ExternalInput").ap()
    wvT = nc.dram_tensor("wvT", [D, D], bf, kind="ExternalInput").ap()
    woT = nc.dram_tensor("woT", [D, D], bf, kind="ExternalInput").ap()
    bo = nc.dram_tensor("bo", [1, D], f32, kind="ExternalInput").ap()
    if need_mask:
        maskb = nc.dram_tensor("maskb", [P, NCHUNK], f32,
                               kind="ExternalInput").ap()
    y = nc.dram_tensor("y", [nq, D], f32, kind="ExternalOutput").ap()
    if debug_dumps:
        dq = nc.dram_tensor("dq", [P, DC, nq], bf, kind="ExternalOutput").ap()
        dk = nc.dram_tensor("dk", [P, DC, NKV], bf, kind="ExternalOutput").ap()
        dv = nc.dram_tensor("dv", [P, NCHUNK, HEADS, DIM_HEAD + 1], bf,
                            kind="ExternalOutput").ap()
        do = nc.dram_tensor("do", [P, DC, nq], bf, kind="ExternalOutput").ap()
        dsc = nc.dram_tensor("dsc", [P, GRP, 512], f32,
                             kind="ExternalOutput").ap()
        dpT = nc.dram_tensor("dpT", [P, GRP, 512], bf,
                             kind="ExternalOutput").ap()
        dav = nc.dram_tensor("dav", [DIM_HEAD + 1, 512], f32,
                             kind="ExternalOutput").ap()

    Exp = mybir.ActivationFunctionType.Exp

    with tile.TileContext(nc) as tc:
        with tc.tile_pool(name="big", bufs=1) as big, \
             tc.tile_pool(name="work", bufs=3) as work, \
             tc.tile_pool(name="pTp", bufs=4) as pTp, \
             tc.tile_pool(name="dram", bufs=2, space="DRAM") as dram, \
             tc.tile_pool(name="proj_ps", bufs=2, space="PSUM") as proj_ps, \
             tc.tile_pool(name="score_ps", bufs=2, space="PSUM") as score_ps, \
             tc.tile_pool(name="av_ps", bufs=2, space="PSUM") as av_ps:

            ctx_sb = big.tile([P, DC, NKV], bf, name="ctx_sb")
            x_sb = big.tile([P, DC, nq], bf, name="x_sb")
            wq_sb = big.tile([P, DC, D], bf, name="wq_sb")
            wk_sb = big.tile([P, DC, D], bf, name="wk_sb")
            wv_sb = big.tile([P, DC, D], bf, name="wv_sb")
            wo_sb = big.tile([P, DC, D], bf, name="wo_sb")
            bo_bc = big.tile([P, D], f32, name="bo_bc")
            q_sb = big.tile([P, DC, nq], bf, name="q_sb")
            k_sb = big.tile([P, DC, NKV], bf, name="k_sb")
            v_sb = big.tile([P, NCHUNK, HEADS, DIM_HEAD + 1], bf, name="v_sb")
            o_sb = big.tile([P, DC, nq], bf, name="o_sb")
            if need_mask:
                mb_sb = big.tile([P, NCHUNK], f32, name="mb_sb")

            # ---- input DMAs: few big strided transfers, 2 queues ----
            xTr = xT.rearrange("(c p) n -> p c n", p=P)
            wqr = wqT.rearrange("(c p) n -> p c n", p=P)
            wkr = wkT.rearrange("(c p) n -> p c n", p=P)
            wvr = wvT.rearrange("(c p) n -> p c n", p=P)
            wor = woT.rearrange("(c p) n -> p c n", p=P)
            ctxr = ctxT.rearrange("(c p) n -> p c n", p=P)
            nc.sync.dma_start(x_sb[:], xTr)
            nc.sync.dma_start(wq_sb[:], wqr)
            nc.gpsimd.dma_start(wk_sb[:], wkr)
            nc.gpsimd.dma_start(wv_sb[:], wvr)
            for nt in range(NT):
                eng = nc.sync if nt % 2 == 0 else nc.gpsimd
                eng.dma_start(ctx_sb[:, :, nt * 512:(nt + 1) * 512],
                              ctxr[:, :, nt * 512:(nt + 1) * 512])
            nc.scalar.dma_start(wo_sb[:], wor)
            nc.scalar.dma_start(bo_bc[:], bo.to_broadcast([P, D]))
            if need_mask:
                nc.scalar.dma_start(mb_sb[:], maskb[:])
            nc.vector.memset(v_sb[:, :, :, DIM_HEAD], 1.0)

            # ---- M0: qT[inner, nq] ----
            for ic in range(DC):
                ps = proj_ps.tile([P, 512], f32, name="ps_proj", tag="proj")
                for kc in range(DC):
                    nc.tensor.matmul(
                        ps[:, :nq], wq_sb[:, kc, ic * P:(ic + 1) * P],
                        x_sb[:, kc, :], start=(kc == 0), stop=(kc == DC - 1))
                nc.vector.tensor_copy(out=q_sb[:, ic, :], in_=ps[:, :nq])

            def emit_m1(ic, nt):
                ps = proj_ps.tile([P, 512], f32, name="ps_proj", tag="proj")
                for kc in range(DC):
                    nc.tensor.matmul(
                        ps, wk_sb[:, kc, ic * P:(ic + 1) * P],
                        ctx_sb[:, kc, nt * 512:(nt + 1) * 512],
                        start=(kc == 0), stop=(kc == DC - 1))
                nc.vector.tensor_copy(
                    out=k_sb[:, ic, nt * 512:(nt + 1) * 512], in_=ps)

            def emit_v(j, hp):
                # v columns for head pair hp only: [128kv, 128]
                ps = proj_ps.tile([P, 512], f32, name="ps_proj", tag="proj")
                for kc in range(DC):
                    nc.tensor.matmul(
                        ps[:, 0:P], ctx_sb[:, kc, j * P:(j + 1) * P],
                        wv_sb[:, kc, hp * P:(hp + 1) * P],
                        start=(kc == 0), stop=(kc == DC - 1))
                nc.vector.tensor_copy(
                    out=v_sb[:, j, 2 * hp:2 * hp + 2, 0:DIM_HEAD],
                    in_=ps[:, 0:P].rearrange("p (h d) -> p h d", h=2))

            # ---- attention: head pairs (even head rows 0-63, odd 64-127) --
            # AV matmuls lag scores/exp by SKEW chunks so the in-order PE
            # stream never blocks on the av-buffer WAR at pair boundaries.
            SKEW = 2
            for hp in range(HEADS // 2):
                ic = hp
                h0, h1 = 2 * hp, 2 * hp + 1
                av0 = av_ps.tile([DIM_HEAD + 1, 512], f32, name="av0",
                                 tag="av")
                av1 = av_ps.tile([DIM_HEAD + 1, 512], f32, name="av1",
                                 tag="av")
                pend = []

                def emit_av(j, pT):
                    nc.tensor.matmul(
                        av0[:, :nq], v_sb[:, j, h0, :], pT[:, 0, :nq],
                        start=(j == 0), stop=(j == NCHUNK - 1))
                    nc.tensor.matmul(
                        av1[:, :nq], v_sb[:, j, h1, :], pT[:, 1, :nq],
                        start=(j == 0), stop=(j == NCHUNK - 1))

                for j in range(NCHUNK):
                    emit_v(j, hp)
                    if hp == 0 and j % 4 == 0:
                        emit_m1(0, j // 4)
                    if hp < HEADS // 2 - 1 and j % 4 == 2:
                        emit_m1(hp + 1, j // 4)

                    sc = score_ps.tile([P, 2, 512], f32, name="sc", tag="sc")
                    nc.tensor.matmul(
                        sc[:, 0, :nq],
                        k_sb[0:DIM_HEAD, ic, j * P:(j + 1) * P],
                        q_sb[0:DIM_HEAD, ic, :], start=True, stop=True)
                    nc.tensor.matmul(
                        sc[:, 1, :nq],
                        k_sb[DIM_HEAD:P, ic, j * P:(j + 1) * P],
                        q_sb[DIM_HEAD:P, ic, :], start=True, stop=True)
                    if debug_dumps and hp == 0 and j == 0:
                        stg = work.tile([P, 2, 512], f32, name="stg",
                                        tag="stg")
                        nc.vector.tensor_copy(out=stg[:], in_=sc[:])
                        nc.sync.dma_start(dsc[:], stg[:])
                    pT = pTp.tile([P, 2, 512], bf, name="pT", tag="pT")
                    if need_mask:
                        nc.scalar.activation(
                            pT[:, :, :nq], sc[:, :, :nq], Exp,
                            bias=mb_sb[:, j, None], scale=0.125)
                    else:
                        nc.scalar.activation(
                            pT[:, :, :nq], sc[:, :, :nq], Exp, scale=0.125)
                    if debug_dumps and hp == 0 and j == 0:
                        nc.sync.dma_start(dpT[:], pT[:])
                    pend.append((j, pT))
                    if len(pend) > SKEW:
                        emit_av(*pend.pop(0))
                for item in pend:
                    emit_av(*item)
                if debug_dumps and hp == 0:
                    stg2 = work.tile([DIM_HEAD + 1, 512], f32, name="stg2",
                                     tag="stg2")
                    nc.vector.tensor_copy(out=stg2[:], in_=av0[:])
                    nc.sync.dma_start(dav[:], stg2[:])
                # softmax normalize: o = av[:64] * recip(av[64])
                for po, av in ((0, av0), (DIM_HEAD, av1)):
                    rec = work.tile([DIM_HEAD + 1, 512], f32, name="rec",
                                    tag="rec")
                    nc.vector.reciprocal(rec[DIM_HEAD:DIM_HEAD + 1, :nq],
                                         av[DIM_HEAD:DIM_HEAD + 1, :nq])
                    rec_dr = dram.tile([1, 512], f32, name="rec_dr",
                                       tag="rec_dr")
                    nc.gpsimd.dma_start(rec_dr[:, :nq],
                                        rec[DIM_HEAD:DIM_HEAD + 1, :nq])
                    rec_bc = work.tile([DIM_HEAD, 512], f32, name="rec_bc",
                                       tag="rec_bc")
                    nc.gpsimd.dma_start(
                        rec_bc[:, :nq],
                        rec_dr[:, :nq].to_broadcast([DIM_HEAD, nq]))
                    o_tmp = work.tile([DIM_HEAD, 512], bf, name="o_tmp",
                                      tag="o_tmp")
                    nc.vector.tensor_mul(o_tmp[:, :nq], av[0:DIM_HEAD, :nq],
                                         rec_bc[:, :nq])
                    nc.sync.dma_start(o_sb[po:po + DIM_HEAD, ic, :],
                                      o_tmp[:, :nq])

            if debug_dumps:
                nc.sync.dma_start(dq[:], q_sb[:])
                nc.sync.dma_start(dk[:], k_sb[:])
                nc.sync.dma_start(dv[:], v_sb[:])
                nc.sync.dma_start(do[:], o_sb[:])

            # ---- M5: y = O @ Wo^T + bo ----
            for qc in range(nq // P):
                ps = proj_ps.tile([P, 512], f32, name="ps_proj", tag="proj")
                for ic in range(DC):
                    nc.tensor.matmul(
                        ps, o_sb[:, ic, qc * P:(qc + 1) * P],
                        wo_sb[:, ic, :], start=(ic == 0), stop=(ic == DC - 1))
                y_sb = work.tile([P, D], f32, name="y_sb", tag="y_sb")
                nc.vector.tensor_add(y_sb, ps, bo_bc)
                nc.sync.dma_start(y[qc * P:(qc + 1) * P, :], y_sb)

    nc.compile()
    return nc


def _get_program(need_mask: bool):
    if need_mask not in _PROGRAMS:
        _PROGRAMS[need_mask] = _build(need_mask)
    return _PROGRAMS[need_mask]


def _prep_inputs(x, context, mask, Wq, Wkv, Wo, bo):
    """Host-side shard + transpose + cast. Returns list of per-core in_maps."""
    x = np.asarray(x, dtype=np.float32)
    context = np.asarray(context, dtype=np.float32)
    mask = np.asarray(mask)
    Wq = np.asarray(Wq, dtype=np.float32)
    Wkv = np.asarray(Wkv, dtype=np.float32)
    Wo = np.asarray(Wo, dtype=np.float32)
    bo = np.asarray(bo, dtype=np.float32)

    need_mask = not bool(mask.all())
    wqT = np.ascontiguousarray(Wq.T).astype(BF16)
    wkT = np.ascontiguousarray(Wkv[:D].T).astype(BF16)
    wvT = np.ascontiguousarray(Wkv[D:].T).astype(BF16)
    woT = np.ascontiguousarray(Wo.T).astype(BF16)
    bo2 = bo.reshape(1, D)

    ctxTs = [np.ascontiguousarray(context[b].T).astype(BF16)
             for b in range(B)]
    if need_mask:
        # additive pre-exp bias: 0 where visible, -1e30 where masked
        mb = [np.where(mask[b], 0.0, -1e30).astype(np.float32)
              .reshape(NCHUNK, P).T.copy() for b in range(B)]

    in_maps = []
    for c in range(N_CORES):
        b, qh = divmod(c, 2)
        qs = qh * NQ_SH
        m = {
            "xT": np.ascontiguousarray(x[b, qs:qs + NQ_SH, :].T).astype(BF16),
            "ctxT": ctxTs[b],
            "wqT": wqT, "wkT": wkT, "wvT": wvT, "woT": woT,
            "bo": bo2,
        }
        if need_mask:
            m["maskb"] = mb[b]
        in_maps.append(m)
    return in_maps, need_mask


def run_sharded(inputs, trace=False):
    """Run on 8 cores; returns (full_output, BassKernelResults)."""
    from concourse import bass_utils
    in_maps, need_mask = _prep_inputs(**inputs)
    nc = _get_program(need_mask)
    res = bass_utils.run_bass_kernel_spmd(
        nc, in_maps, core_ids=list(range(N_CORES)), trace=trace)
    out = np.empty((B, NQ, D), dtype=np.float32)
    for c in range(N_CORES):
        b, qh = divmod(c, 2)
        qs = qh * NQ_SH
        out[b, qs:qs + NQ_SH, :] = res.results[c]["y"]
    return out, res


def kernel(**inputs) -> np.ndarray:
    out, _ = run_sharded(inputs, trace=False)
    return out



# revision 5
# speedup vs baseline: 1.0572x; 1.0572x over previous
"""Cross-attention Trainium2 kernel (nn_CrossAttention_7627861918199).

Full-input contract: kernel(**inputs) takes the unsharded numpy inputs and
returns the full [B, NQ, D] float32 output.

Sharding: 8 cores = (batch b, head-group hg); core c handles batch c//2 and
heads [4*(c%2), 4*(c%2)+4) for ALL nq=1024 queries.  Tensor-parallel over
heads: Wq/Wkv are split column-wise (256 inner dims per core), Wo row-wise;
each core emits a partial y = O_hg @ Wo_hg^T and the host sums the two
partials per batch during unshard (the "all-reduce after to_out").  This
halves the K/V projection work vs. query-sharding (context is projected
once per head-group, not once per query-half).

Per-core pipeline (all matmuls bf16, fp32 accumulate):
  qT = Wq_hg @ xT          [256, 1024]
  kT = Wk_hg @ ctxT        [256, 4096]
  v  = ctx @ Wv_hg^T       [4096, 4x65]  (65th col = ones for softmax sums)
  4 segments (head-pair hp x query-half qh), 32 kv-chunks each:
    S^T chunk [128kv, 2, 512q] = k @ qT  -> exp (ACT, scale=1/8) -> P^T bf16
    -> av[65, 512] += v_aug^T @ P^T      (PSUM accum over 32 chunks)
    segment end: stage av PSUM->SBUF (frees banks), batched reciprocal of
    the two sum rows, DMA-broadcast, normalize, write O^T
  y_partial = O^T.T @ Wo_hg^T            (bo added on host)
"""

import numpy as np
import ml_dtypes

HEADS = 8
DIM_HEAD = 64
D = 512          # QUERY_DIM == full inner dim
B, NQ, NKV = 4, 1024, 4096
N_CORES = 8
NHL = 4          # heads per core
INNER = NHL * DIM_HEAD       # 256 local inner dims
P = 128
KC = D // P                  # 4 contraction chunks of 128 (over QUERY_DIM)
ICK = INNER // P             # 2 local inner chunks of 128
NCHUNK = NKV // P            # 32 kv chunks of 128
NT = NKV // 512              # 8 n-tiles for kT
NQH = NQ // 512              # 2 query halves
BF16 = ml_dtypes.bfloat16

_PROGRAMS = {}


def _build(need_mask: bool, num_devices: int = N_CORES):
    import concourse.mybir as mybir
    import concourse.tile as tile
    from concourse import bacc

    dt = mybir.dt
    f32, bf = dt.float32, dt.bfloat16

    nc = bacc.Bacc("TRN2", target_bir_lowering=False, debug=False,
                   num_devices=num_devices)

    xT = nc.dram_tensor("xT", [D, NQ], bf, kind="ExternalInput").ap()
    ctxT = nc.dram_tensor("ctxT", [D, NKV], bf, kind="ExternalInput").ap()
    wqT = nc.dram_tensor("wqT", [D, INNER], bf, kind="ExternalInput").ap()
    wkT = nc.dram_tensor("wkT", [D, INNER], bf, kind="ExternalInput").ap()
    wvT = nc.dram_tensor("wvT", [D, INNER], bf, kind="ExternalInput").ap()
    woT = nc.dram_tensor("woT", [INNER, D], bf, kind="ExternalInput").ap()
    if need_mask:
        maskb = nc.dram_tensor("maskb", [P, NCHUNK], f32,
                               kind="ExternalInput").ap()
    y = nc.dram_tensor("y", [NQ, D], f32, kind="ExternalOutput").ap()

    Exp = mybir.ActivationFunctionType.Exp

    with tile.TileContext(nc) as tc:
        with tc.tile_pool(name="big", bufs=1) as big, \
             tc.tile_pool(name="work", bufs=3) as work, \
             tc.tile_pool(name="pTp", bufs=4) as pTp, \
             tc.tile_pool(name="dram", bufs=2, space="DRAM") as dram, \
             tc.tile_pool(name="proj_ps", bufs=2, space="PSUM") as proj_ps, \
             tc.tile_pool(name="score_ps", bufs=2, space="PSUM") as score_ps, \
             tc.tile_pool(name="av_ps", bufs=2, space="PSUM") as av_ps:

            ctx_sb = big.tile([P, KC, NKV], bf, name="ctx_sb")
            x_sb = big.tile([P, KC, NQ], bf, name="x_sb")
            wq_sb = big.tile([P, KC, INNER], bf, name="wq_sb")
            wk_sb = big.tile([P, KC, INNER], bf, name="wk_sb")
            wv_sb = big.tile([P, KC, INNER], bf, name="wv_sb")
            wo_sb = big.tile([P, ICK, D], bf, name="wo_sb")
            q_sb = big.tile([P, ICK, NQ], bf, name="q_sb")
            k_sb = big.tile([P, ICK, NKV], bf, name="k_sb")
            v_sb = big.tile([P, NCHUNK, NHL, DIM_HEAD + 1], bf, name="v_sb")
            o_sb = big.tile([P, ICK, NQ], bf, name="o_sb")
            if need_mask:
                mb_sb = big.tile([P, NCHUNK], f32, name="mb_sb")

            # ---- input DMAs: few big strided transfers, 2 queues ----
            xTr = xT.rearrange("(c p) n -> p c n", p=P)
            wqr = wqT.rearrange("(c p) n -> p c n", p=P)
            wkr = wkT.rearrange("(c p) n -> p c n", p=P)
            wvr = wvT.rearrange("(c p) n -> p c n", p=P)
            wor = woT.rearrange("(c p) n -> p c n", p=P)
            ctxr = ctxT.rearrange("(c p) n -> p c n", p=P)
            nc.sync.dma_start(x_sb[:], xTr)
            nc.sync.dma_start(wq_sb[:], wqr)
            nc.gpsimd.dma_start(wk_sb[:], wkr)
            nc.gpsimd.dma_start(wv_sb[:], wvr)
            for nt in range(NT):
                eng = nc.sync if nt % 2 == 0 else nc.gpsimd
                eng.dma_start(ctx_sb[:, :, nt * 512:(nt + 1) * 512],
                              ctxr[:, :, nt * 512:(nt + 1) * 512])
            nc.scalar.dma_start(wo_sb[:], wor)
            if need_mask:
                nc.scalar.dma_start(mb_sb[:], maskb[:])
            nc.vector.memset(v_sb[:, :, :, DIM_HEAD], 1.0)

            # ---- Q proj: qT[256, 1024] ----
            for ic in range(ICK):
                for qh in range(NQH):
                    ps = proj_ps.tile([P, 512], f32, name="ps_proj",
                                      tag="proj")
                    for kc in range(KC):
                        nc.tensor.matmul(
                            ps, wq_sb[:, kc, ic * P:(ic + 1) * P],
                            x_sb[:, kc, qh * 512:(qh + 1) * 512],
                            start=(kc == 0), stop=(kc == KC - 1))
                    nc.vector.tensor_copy(
                        out=q_sb[:, ic, qh * 512:(qh + 1) * 512], in_=ps)

            def emit_k(ic, nt):
                ps = proj_ps.tile([P, 512], f32, name="ps_proj", tag="proj")
                for kc in range(KC):
                    nc.tensor.matmul(
                        ps, wk_sb[:, kc, ic * P:(ic + 1) * P],
                        ctx_sb[:, kc, nt * 512:(nt + 1) * 512],
                        start=(kc == 0), stop=(kc == KC - 1))
                nc.vector.tensor_copy(
                    out=k_sb[:, ic, nt * 512:(nt + 1) * 512], in_=ps)

            def emit_v(j):
                # v rows for kv chunk j, all 4 local heads: [128kv, 256]
                ps = proj_ps.tile([P, 512], f32, name="ps_proj", tag="proj")
                for kc in range(KC):
                    nc.tensor.matmul(
                        ps[:, 0:INNER], ctx_sb[:, kc, j * P:(j + 1) * P],
                        wv_sb[:, kc, :], start=(kc == 0), stop=(kc == KC - 1))
                nc.vector.tensor_copy(
                    out=v_sb[:, j, :, 0:DIM_HEAD],
                    in_=ps[:, 0:INNER].rearrange("p (h d) -> p h d", h=NHL))

            def emit_oproj(qc):
                # y_partial chunk = O[qc] @ Wo_hg^T
                ps = proj_ps.tile([P, 512], f32, name="ps_proj", tag="proj")
                for ic in range(ICK):
                    nc.tensor.matmul(
                        ps, o_sb[:, ic, qc * P:(qc + 1) * P],
                        wo_sb[:, ic, :], start=(ic == 0), stop=(ic == ICK - 1))
                y_sb = work.tile([P, D], f32, name="y_sb", tag="y_sb")
                nc.vector.tensor_copy(out=y_sb, in_=ps)
                nc.sync.dma_start(y[qc * P:(qc + 1) * P, :], y_sb)

            # prologue: k ic0 nt0/nt1 and v chunks 0,1 so segment (0,0)
            # never waits at its head.
            emit_k(0, 0)
            emit_v(0)
            emit_k(0, 1)
            emit_v(1)

            SKEW = 2
            for hp in range(ICK):          # head pair = inner chunk
                h0, h1 = 2 * hp, 2 * hp + 1
                for qh in range(NQH):
                    seg0 = (hp == 0 and qh == 0)
                    seg1 = (hp == 0 and qh == 1)
                    seg3 = (hp == 1 and qh == 1)
                    av0 = av_ps.tile([DIM_HEAD + 1, 512], f32, name="av0",
                                     tag="av")
                    av1 = av_ps.tile([DIM_HEAD + 1, 512], f32, name="av1",
                                     tag="av")
                    pend = []

                    def emit_av(j, pT, av0=av0, av1=av1, h0=h0, h1=h1):
                        nc.tensor.matmul(
                            av0, v_sb[:, j, h0, :], pT[:, 0, :],
                            start=(j == 0), stop=(j == NCHUNK - 1))
                        nc.tensor.matmul(
                            av1, v_sb[:, j, h1, :], pT[:, 1, :],
                            start=(j == 0), stop=(j == NCHUNK - 1))

                    for j in range(NCHUNK):
                        if seg0:
                            if j + 2 < NCHUNK:
                                emit_v(j + 2)
                            if j % 4 == 2 and j < 24:
                                emit_k(0, j // 4 + 2)
                        if seg1:
                            if j % 4 == 1:
                                emit_k(1, j // 4)
                        if seg3 and j in (6, 8, 10, 12):
                            emit_oproj((j - 6) // 2)

                        sc = score_ps.tile([P, 2, 512], f32, name="sc",
                                           tag="sc")
                        nc.tensor.matmul(
                            sc[:, 0, :],
                            k_sb[0:DIM_HEAD, hp, j * P:(j + 1) * P],
                            q_sb[0:DIM_HEAD, hp, qh * 512:(qh + 1) * 512],
                            start=True, stop=True)
                        nc.tensor.matmul(
                            sc[:, 1, :],
                            k_sb[DIM_HEAD:P, hp, j * P:(j + 1) * P],
                            q_sb[DIM_HEAD:P, hp, qh * 512:(qh + 1) * 512],
                            start=True, stop=True)
                        pT = pTp.tile([P, 2, 512], bf, name="pT", tag="pT")
                        if need_mask:
                            nc.scalar.activation(
                                pT[:], sc[:], Exp,
                                bias=mb_sb[:, j, None], scale=0.125)
                        else:
                            nc.scalar.activation(pT[:], sc[:], Exp,
                                                 scale=0.125)
                        pend.append((j, pT))
                        if len(pend) > SKEW:
                            emit_av(*pend.pop(0))
                    for item in pend:
                        emit_av(*item)

                    # ---- normalize: stage PSUM->SBUF, batched recip ----
                    avs = work.tile([DIM_HEAD + 1, 2, 512], f32, name="avs",
                                    tag="avs")
                    nc.vector.tensor_copy(out=avs[:, 0, :], in_=av0[:])
                    nc.vector.tensor_copy(out=avs[:, 1, :], in_=av1[:])
                    zr = work.tile([2, 512], f32, name="zr", tag="zr")
                    nc.sync.dma_start(zr[0:1, :],
                                      avs[DIM_HEAD:DIM_HEAD + 1, 0, :])
                    nc.sync.dma_start(zr[1:2, :],
                                      avs[DIM_HEAD:DIM_HEAD + 1, 1, :])
                    rec = work.tile([2, 512], f32, name="rec", tag="rec")
                    nc.vector.reciprocal(rec[:], zr[:])
                    rec_dr = dram.tile([2, 512], f32, name="rec_dr",
                                       tag="rec_dr")
                    nc.gpsimd.dma_start(rec_dr[:], rec[:])
                    rec_bc = work.tile([DIM_HEAD, 2, 512], f32, name="rec_bc",
                                       tag="rec_bc")
                    nc.gpsimd.dma_start(
                        rec_bc[:, 0, :],
                        rec_dr[0:1, :].to_broadcast([DIM_HEAD, 512]))
                    nc.gpsimd.dma_start(
                        rec_bc[:, 1, :],
                        rec_dr[1:2, :].to_broadcast([DIM_HEAD, 512]))
                    o_tmp = work.tile([DIM_HEAD, 2, 512], bf, name="o_tmp",
                                      tag="o_tmp")
                    nc.vector.tensor_mul(o_tmp[:, 0, :],
                                         avs[0:DIM_HEAD, 0, :],
                                         rec_bc[:, 0, :])
                    nc.vector.tensor_mul(o_tmp[:, 1, :],
                                         avs[0:DIM_HEAD, 1, :],
                                         rec_bc[:, 1, :])
                    nc.sync.dma_start(
                        o_sb[0:DIM_HEAD, hp, qh * 512:(qh + 1) * 512],
                        o_tmp[:, 0, :])
                    nc.sync.dma_start(
                        o_sb[DIM_HEAD:P, hp, qh * 512:(qh + 1) * 512],
                        o_tmp[:, 1, :])

            # ---- y_partial qc 4-7 (qc 0-3 emitted inside seg3) ----
            for qc in range(4, NQ // P):
                emit_oproj(qc)

    nc.compile()
    return nc


def _get_program(need_mask: bool):
    if need_mask not in _PROGRAMS:
        _PROGRAMS[need_mask] = _build(need_mask)
    return _PROGRAMS[need_mask]


def _prep_inputs(x, context, mask, Wq, Wkv, Wo, bo):
    """Host-side shard + transpose + cast. Returns (in_maps, need_mask, bo)."""
    x = np.asarray(x, dtype=np.float32)
    context = np.asarray(context, dtype=np.float32)
    mask = np.asarray(mask)
    Wq = np.asarray(Wq, dtype=np.float32)
    Wkv = np.asarray(Wkv, dtype=np.float32)
    Wo = np.asarray(Wo, dtype=np.float32)
    bo = np.asarray(bo, dtype=np.float32)

    need_mask = not bool(mask.all())
    xTs = [np.ascontiguousarray(x[b].T).astype(BF16) for b in range(B)]
    ctxTs = [np.ascontiguousarray(context[b].T).astype(BF16)
             for b in range(B)]
    wqTs, wkTs, wvTs, woTs = [], [], [], []
    for hg in range(2):
        sl = slice(hg * INNER, (hg + 1) * INNER)
        wqTs.append(np.ascontiguousarray(Wq[sl].T).astype(BF16))
        wkTs.append(np.ascontiguousarray(Wkv[:D][sl].T).astype(BF16))
        wvTs.append(np.ascontiguousarray(Wkv[D:][sl].T).astype(BF16))
        woTs.append(np.ascontiguousarray(Wo[:, sl].T).astype(BF16))
    if need_mask:
        # additive pre-exp bias: 0 where visible, -1e30 where masked
        mb = [np.where(mask[b], 0.0, -1e30).astype(np.float32)
              .reshape(NCHUNK, P).T.copy() for b in range(B)]

    in_maps = []
    for c in range(N_CORES):
        b, hg = divmod(c, 2)
        m = {
            "xT": xTs[b], "ctxT": ctxTs[b],
            "wqT": wqTs[hg], "wkT": wkTs[hg], "wvT": wvTs[hg],
            "woT": woTs[hg],
        }
        if need_mask:
            m["maskb"] = mb[b]
        in_maps.append(m)
    return in_maps, need_mask, bo


def run_sharded(inputs, trace=False):
    """Run on 8 cores; returns (full_output, BassKernelResults)."""
    from concourse import bass_utils
    in_maps, need_mask, bo = _prep_inputs(**inputs)
    nc = _get_program(need_mask)
    res = bass_utils.run_bass_kernel_spmd(
        nc, in_maps, core_ids=list(range(N_CORES)), trace=trace)
    out = np.empty((B, NQ, D), dtype=np.float32)
    for b in range(B):
        out[b] = res.results[2 * b]["y"]
        out[b] += res.results[2 * b + 1]["y"]
        out[b] += bo
    return out, res


def kernel(**inputs) -> np.ndarray:
    out, _ = run_sharded(inputs, trace=False)
    return out


# revision 14
# speedup vs baseline: 1.1778x; 1.1141x over previous
"""Cross-attention Trainium2 kernel (nn_CrossAttention_7627861918199).

Full-input contract: kernel(**inputs) takes the unsharded numpy inputs and
returns the full [B, NQ, D] float32 output.

Sharding: 8 cores = (batch b, head-group hg); core c handles batch c//2 and
heads [4*(c%2), 4*(c%2)+4) for ALL nq=1024 queries.  Tensor-parallel over
heads: Wq/Wkv are split column-wise (256 inner dims per core), Wo row-wise;
each core emits a partial y = O_hg @ Wo_hg^T and the host sums the two
partials per batch during unshard (the "all-reduce after to_out").  This
halves the K/V projection work vs. query-sharding (context is projected
once per head-group, not once per query-half).

Per-core pipeline (all matmuls bf16, fp32 accumulate):
  qT = Wq_hg @ xT          [256, 1024]
  kT = Wk_hg @ ctxT        [256, 4096]
  v  = ctx @ Wv_hg^T       [4096, 4x65]  (65th col = ones for softmax sums)
  4 segments (head-pair hp x query-half qh), 32 kv-chunks each:
    S^T chunk [128kv, 1024] = k @ qT  -> exp (ACT, scale=1/8) -> P^T bf16
    -> av[65, 512] += v_aug^T @ P^T   (PSUM accum over 32 chunks)
    segment end: stage av PSUM->SBUF (frees banks), fast-reciprocal of the
    two sum rows in place, DMA-broadcast, normalize, write O^T
  y_partial = O^T.T @ Wo_hg^T            (bo added on host)
Input DMAs are spread across 5 engine queues; projections are interleaved
into segments with PE slack (V+Kic0 in seg0, Kic1 split over seg1/seg2,
y projection for the first query-half inside seg3).
"""

import numpy as np
import ml_dtypes

HEADS = 8
DIM_HEAD = 64
D = 512          # QUERY_DIM == full inner dim
B, NQ, NKV = 4, 1024, 4096
N_CORES = 8
NHL = 4          # heads per core
INNER = NHL * DIM_HEAD       # 256 local inner dims
P = 128
KC = D // P                  # 4 contraction chunks of 128 (over QUERY_DIM)
ICK = INNER // P             # 2 local inner chunks of 128
NCHUNK = NKV // P            # 32 kv chunks of 128
NT = NKV // 512              # 8 n-tiles for kT
NQH = NQ // 512              # 2 query halves
BF16 = ml_dtypes.bfloat16

_PROGRAMS = {}


def _build(need_mask: bool, num_devices: int = N_CORES):
    import concourse.mybir as mybir
    import concourse.tile as tile
    from concourse import bacc

    dt = mybir.dt
    f32, bf = dt.float32, dt.bfloat16

    nc = bacc.Bacc("TRN2", target_bir_lowering=False, debug=False,
                   num_devices=num_devices)

    xT = nc.dram_tensor("xT", [D, NQ], bf, kind="ExternalInput").ap()
    ctxT = nc.dram_tensor("ctxT", [D, NKV], bf, kind="ExternalInput").ap()
    wqT = nc.dram_tensor("wqT", [D, INNER], bf, kind="ExternalInput").ap()
    wkT = nc.dram_tensor("wkT", [D, INNER], bf, kind="ExternalInput").ap()
    wvT = nc.dram_tensor("wvT", [D, INNER], bf, kind="ExternalInput").ap()
    woT = nc.dram_tensor("woT", [INNER, D], bf, kind="ExternalInput").ap()
    if need_mask:
        maskb = nc.dram_tensor("maskb", [P, NCHUNK], f32,
                               kind="ExternalInput").ap()
    y = nc.dram_tensor("y", [NQ, D], f32, kind="ExternalOutput").ap()

    Exp = mybir.ActivationFunctionType.Exp

    with tile.TileContext(nc) as tc:
        with tc.tile_pool(name="big", bufs=1) as big, \
             tc.tile_pool(name="work", bufs=3) as work, \
             tc.tile_pool(name="pTp", bufs=6) as pTp, \
             tc.tile_pool(name="dram", bufs=2, space="DRAM") as dram, \
             tc.tile_pool(name="proj_ps", bufs=2, space="PSUM") as proj_ps, \
             tc.tile_pool(name="score_ps", bufs=2, space="PSUM") as score_ps, \
             tc.tile_pool(name="av_ps", bufs=2, space="PSUM") as av_ps:

            ctx_sb = big.tile([P, KC, NKV], bf, name="ctx_sb")
            x_sb = big.tile([P, KC, NQ], bf, name="x_sb")
            wq_sb = big.tile([P, KC, INNER], bf, name="wq_sb")
            wk_sb = big.tile([P, KC, INNER], bf, name="wk_sb")
            wv_sb = big.tile([P, KC, INNER], bf, name="wv_sb")
            wo_sb = big.tile([P, ICK, D], bf, name="wo_sb")
            q_sb = big.tile([P, ICK, NQ], bf, name="q_sb")
            k_sb = big.tile([P, ICK, NKV], bf, name="k_sb")
            v_sb = big.tile([P, NCHUNK, NHL, DIM_HEAD + 1], bf, name="v_sb")
            o_sb = big.tile([P, ICK, NQ], bf, name="o_sb")
            if need_mask:
                mb_sb = big.tile([P, NCHUNK], f32, name="mb_sb")

            # ---- input DMAs spread over 5 engine queues ----
            xTr = xT.rearrange("(c p) n -> p c n", p=P)
            wqr = wqT.rearrange("(c p) n -> p c n", p=P)
            wkr = wkT.rearrange("(c p) n -> p c n", p=P)
            wvr = wvT.rearrange("(c p) n -> p c n", p=P)
            wor = woT.rearrange("(c p) n -> p c n", p=P)
            ctxr = ctxT.rearrange("(c p) n -> p c n", p=P)
            nc.scalar.dma_start(wq_sb[:], wqr)
            nc.gpsimd.dma_start(wk_sb[:], wkr)
            nc.scalar.dma_start(wv_sb[:], wvr)
            nc.sync.dma_start(x_sb[:], xTr)
            ctx_eng = [nc.gpsimd, nc.gpsimd, nc.sync, nc.gpsimd,
                       nc.gpsimd, nc.sync, nc.scalar, nc.scalar]
            for nt in range(NT):
                ctx_eng[nt].dma_start(ctx_sb[:, :, nt * 512:(nt + 1) * 512],
                                      ctxr[:, :, nt * 512:(nt + 1) * 512])
            nc.scalar.dma_start(wo_sb[:], wor)
            if need_mask:
                nc.scalar.dma_start(mb_sb[:], maskb[:])
            nc.vector.memset(v_sb[:, :, :, DIM_HEAD], 1.0)

            # ---- Q proj: qT[256, 1024] ----
            for ic in range(ICK):
                for qh in range(NQH):
                    ps = proj_ps.tile([P, 512], f32, name="ps_proj",
                                      tag="proj")
                    for kc in range(KC):
                        nc.tensor.matmul(
                            ps, wq_sb[:, kc, ic * P:(ic + 1) * P],
                            x_sb[:, kc, qh * 512:(qh + 1) * 512],
                            start=(kc == 0), stop=(kc == KC - 1))
                    nc.vector.tensor_copy(
                        out=q_sb[:, ic, qh * 512:(qh + 1) * 512], in_=ps)

            def emit_k(ic, nt):
                ps = proj_ps.tile([P, 512], f32, name="ps_proj", tag="proj")
                for kc in range(KC):
                    nc.tensor.matmul(
                        ps, wk_sb[:, kc, ic * P:(ic + 1) * P],
                        ctx_sb[:, kc, nt * 512:(nt + 1) * 512],
                        start=(kc == 0), stop=(kc == KC - 1))
                nc.vector.tensor_copy(
                    out=k_sb[:, ic, nt * 512:(nt + 1) * 512], in_=ps)

            def emit_v(j):
                # v rows for kv chunk j, all 4 local heads: [128kv, 256]
                ps = proj_ps.tile([P, 512], f32, name="ps_proj", tag="proj")
                for kc in range(KC):
                    nc.tensor.matmul(
                        ps[:, 0:INNER], ctx_sb[:, kc, j * P:(j + 1) * P],
                        wv_sb[:, kc, :], start=(kc == 0), stop=(kc == KC - 1))
                nc.vector.tensor_copy(
                    out=v_sb[:, j, :, 0:DIM_HEAD],
                    in_=ps[:, 0:INNER].rearrange("p (h d) -> p h d", h=NHL))

            y_eng = [nc.gpsimd, nc.sync, nc.gpsimd, nc.sync,
                     nc.scalar, nc.sync, nc.gpsimd, nc.scalar]

            def emit_oproj(qc):
                # y_partial chunk = O[qc] @ Wo_hg^T
                ps = proj_ps.tile([P, 512], f32, name="ps_proj", tag="proj")
                for ic in range(ICK):
                    nc.tensor.matmul(
                        ps, o_sb[:, ic, qc * P:(qc + 1) * P],
                        wo_sb[:, ic, :], start=(ic == 0), stop=(ic == ICK - 1))
                y_sb = work.tile([P, D], f32, name="y_sb", tag="y_sb")
                nc.vector.tensor_copy(out=y_sb, in_=ps)
                y_eng[qc].dma_start(y[qc * P:(qc + 1) * P, :], y_sb)

            # prologue: k ic0 nt0/nt1 and v chunks 0,1 so segment (0,0)
            # never waits at its head.
            emit_k(0, 0)
            emit_v(0)
            emit_k(0, 1)
            emit_v(1)

            SKEW = 3
            for hp in range(ICK):          # head pair = inner chunk
                h0, h1 = 2 * hp, 2 * hp + 1
                for qh in range(NQH):
                    seg0 = (hp == 0 and qh == 0)
                    seg1 = (hp == 0 and qh == 1)
                    seg2 = (hp == 1 and qh == 0)
                    seg3 = (hp == 1 and qh == 1)
                    av0 = av_ps.tile([DIM_HEAD + 1, 512], f32, name="av0",
                                     tag="av")
                    av1 = av_ps.tile([DIM_HEAD + 1, 512], f32, name="av1",
                                     tag="av")
                    pend = []

                    def emit_av(j, pT, av0=av0, av1=av1, h0=h0, h1=h1):
                        nc.tensor.matmul(
                            av0, v_sb[:, j, h0, :], pT[:, 0:512],
                            start=(j == 0), stop=(j == NCHUNK - 1))
                        nc.tensor.matmul(
                            av1, v_sb[:, j, h1, :], pT[:, 512:1024],
                            start=(j == 0), stop=(j == NCHUNK - 1))

                    for j in range(NCHUNK):
                        if seg0:
                            if j + 2 < NCHUNK:
                                emit_v(j + 2)
                            if j % 4 == 2 and j < 24:
                                emit_k(0, j // 4 + 2)
                        if seg1 and j in (21, 25):
                            emit_k(1, (j - 21) // 4)
                        if seg2 and j % 4 == 0 and j < 24:
                            emit_k(1, j // 4 + 2)
                        if seg3 and j in (6, 8, 10, 12):
                            emit_oproj((j - 6) // 2)

                        sc = score_ps.tile([P, 1024], f32, name="sc",
                                           tag="sc")
                        nc.tensor.matmul(
                            sc[:, 0:512],
                            k_sb[0:DIM_HEAD, hp, j * P:(j + 1) * P],
                            q_sb[0:DIM_HEAD, hp, qh * 512:(qh + 1) * 512],
                            start=True, stop=True)
                        nc.tensor.matmul(
                            sc[:, 512:1024],
                            k_sb[DIM_HEAD:P, hp, j * P:(j + 1) * P],
                            q_sb[DIM_HEAD:P, hp, qh * 512:(qh + 1) * 512],
                            start=True, stop=True)
                        pT = pTp.tile([P, 1024], bf, name="pT", tag="pT")
                        if need_mask:
                            nc.scalar.activation(
                                pT[:], sc[:], Exp,
                                bias=mb_sb[:, j, None], scale=0.125)
                        else:
                            nc.scalar.activation(pT[:], sc[:], Exp,
                                                 scale=0.125)
                        pend.append((j, pT))
                        if len(pend) > SKEW:
                            emit_av(*pend.pop(0))
                    for item in pend:
                        emit_av(*item)

                    # ---- normalize: stage PSUM->SBUF, in-place fast recip
                    # of the sum rows, DMA-broadcast, scale, write O^T ----
                    avs = work.tile([DIM_HEAD + 1, 2, 512], f32, name="avs",
                                    tag="avs")
                    nc.vector.tensor_copy(out=avs[:, 0, :], in_=av0[:])
                    nc.vector.tensor_copy(out=avs[:, 1, :], in_=av1[:])
                    zrow = avs[DIM_HEAD:DIM_HEAD + 1, :, :]
                    rec = work.tile([DIM_HEAD + 1, 2, 512], f32, name="rec",
                                    tag="rec")
                    rrow = rec[DIM_HEAD:DIM_HEAD + 1, :, :]
                    nc.vector.reciprocal(rrow, zrow)
                    rec_dr = dram.tile([1, 2, 512], f32, name="rec_dr",
                                       tag="rec_dr")
                    nc.gpsimd.dma_start(rec_dr[:], rrow)
                    rec_bc = work.tile([DIM_HEAD, 2, 512], f32, name="rec_bc",
                                       tag="rec_bc")
                    nc.gpsimd.dma_start(
                        rec_bc[:, 0, :],
                        rec_dr[0:1, 0, :].to_broadcast([DIM_HEAD, 512]))
                    nc.gpsimd.dma_start(
                        rec_bc[:, 1, :],
                        rec_dr[0:1, 1, :].to_broadcast([DIM_HEAD, 512]))
                    o_tmp = work.tile([DIM_HEAD, 2, 512], bf, name="o_tmp",
                                      tag="o_tmp")
                    nc.vector.tensor_mul(o_tmp[:, 0, :],
                                         avs[0:DIM_HEAD, 0, :],
                                         rec_bc[:, 0, :])
                    nc.vector.tensor_mul(o_tmp[:, 1, :],
                                         avs[0:DIM_HEAD, 1, :],
                                         rec_bc[:, 1, :])
                    nc.sync.dma_start(
                        o_sb[0:DIM_HEAD, hp, qh * 512:(qh + 1) * 512],
                        o_tmp[:, 0, :])
                    nc.gpsimd.dma_start(
                        o_sb[DIM_HEAD:P, hp, qh * 512:(qh + 1) * 512],
                        o_tmp[:, 1, :])

            # ---- y_partial qc 4-7 (qc 0-3 emitted inside seg3) ----
            for qc in range(4, NQ // P):
                emit_oproj(qc)

    nc.compile()
    return nc


def _get_program(need_mask: bool):
    if need_mask not in _PROGRAMS:
        _PROGRAMS[need_mask] = _build(need_mask)
    return _PROGRAMS[need_mask]


def _prep_inputs(x, context, mask, Wq, Wkv, Wo, bo):
    """Host-side shard + transpose + cast. Returns (in_maps, need_mask, bo)."""
    x = np.asarray(x, dtype=np.float32)
    context = np.asarray(context, dtype=np.float32)
    mask = np.asarray(mask)
    Wq = np.asarray(Wq, dtype=np.float32)
    Wkv = np.asarray(Wkv, dtype=np.float32)
    Wo = np.asarray(Wo, dtype=np.float32)
    bo = np.asarray(bo, dtype=np.float32)

    need_mask = not bool(mask.all())
    xTs = [np.ascontiguousarray(x[b].T).astype(BF16) for b in range(B)]
    ctxTs = [np.ascontiguousarray(context[b].T).astype(BF16)
             for b in range(B)]
    wqTs, wkTs, wvTs, woTs = [], [], [], []
    for hg in range(2):
        sl = slice(hg * INNER, (hg + 1) * INNER)
        wqTs.append(np.ascontiguousarray(Wq[sl].T).astype(BF16))
        wkTs.append(np.ascontiguousarray(Wkv[:D][sl].T).astype(BF16))
        wvTs.append(np.ascontiguousarray(Wkv[D:][sl].T).astype(BF16))
        woTs.append(np.ascontiguousarray(Wo[:, sl].T).astype(BF16))
    if need_mask:
        # additive pre-exp bias: 0 where visible, -1e30 where masked
        mb = [np.where(mask[b], 0.0, -1e30).astype(np.float32)
              .reshape(NCHUNK, P).T.copy() for b in range(B)]

    in_maps = []
    for c in range(N_CORES):
        b, hg = divmod(c, 2)
        m = {
            "xT": xTs[b], "ctxT": ctxTs[b],
            "wqT": wqTs[hg], "wkT": wkTs[hg], "wvT": wvTs[hg],
            "woT": woTs[hg],
        }
        if need_mask:
            m["maskb"] = mb[b]
        in_maps.append(m)
    return in_maps, need_mask, bo


def run_sharded(inputs, trace=False):
    """Run on 8 cores; returns (full_output, BassKernelResults)."""
    from concourse import bass_utils
    in_maps, need_mask, bo = _prep_inputs(**inputs)
    nc = _get_program(need_mask)
    res = bass_utils.run_bass_kernel_spmd(
        nc, in_maps, core_ids=list(range(N_CORES)), trace=trace)
    out = np.empty((B, NQ, D), dtype=np.float32)
    for b in range(B):
        out[b] = res.results[2 * b]["y"]
        out[b] += res.results[2 * b + 1]["y"]
        out[b] += bo
    return out, res


def kernel(**inputs) -> np.ndarray:
    out, _ = run_sharded(inputs, trace=False)
    return out


# revision 20
# speedup vs baseline: 1.2497x; 1.0610x over previous
"""Cross-attention Trainium2 kernel (nn_CrossAttention_7627861918199).

Full-input contract: kernel(**inputs) takes the unsharded numpy inputs and
returns the full [B, NQ, D] float32 output.

Sharding: 8 cores = (batch b, head-group hg); core c handles batch c//2 and
heads [4*(c%2), 4*(c%2)+4) for ALL nq=1024 queries.  Tensor-parallel over
heads: Wq/Wkv are split column-wise (256 inner dims per core), Wo row-wise;
each core emits a partial y = O_hg @ Wo_hg^T and the host sums the two
partials per batch during unshard (the "all-reduce after to_out").  This
halves the K/V projection work vs. query-sharding (context is projected
once per head-group, not once per query-half).

Per-core pipeline (all matmuls bf16, fp32 accumulate):
  qT = Wq_hg @ xT          [256, 1024]
  kT = Wk_hg @ ctxT        [256, 4096]
  v  = ctx @ Wv_hg^T       [4096, 4x65]  (65th col = ones for softmax sums)
  4 segments (head-pair hp x query-half qh), 32 kv-chunks each:
    S^T chunk [128kv, 1024] = k @ qT  -> exp (ACT, scale=1/8) -> P^T bf16
    -> av[65, 512] += v_aug^T @ P^T   (PSUM accum over 32 chunks)
    segment end: stage av PSUM->SBUF (frees banks), fast-reciprocal of the
    two sum rows in place, DMA-broadcast, normalize, write O^T
  y_partial = O^T.T @ Wo_hg^T            (bo added on host)
Input DMAs are spread across 5 engine queues; projections are interleaved
into segments with PE slack (V+Kic0 in seg0, Kic1 split over seg1/seg2,
y projection for the first query-half inside seg3).
"""

import numpy as np
import ml_dtypes

HEADS = 8
DIM_HEAD = 64
D = 512          # QUERY_DIM == full inner dim
B, NQ, NKV = 4, 1024, 4096
N_CORES = 8
NHL = 4          # heads per core
INNER = NHL * DIM_HEAD       # 256 local inner dims
P = 128
KC = D // P                  # 4 contraction chunks of 128 (over QUERY_DIM)
ICK = INNER // P             # 2 local inner chunks of 128
NCHUNK = NKV // P            # 32 kv chunks of 128
NT = NKV // 512              # 8 n-tiles for kT
NQH = NQ // 512              # 2 query halves
BF16 = ml_dtypes.bfloat16

_PROGRAMS = {}


def _build(need_mask: bool, num_devices: int = N_CORES):
    import concourse.mybir as mybir
    import concourse.tile as tile
    from concourse import bacc

    dt = mybir.dt
    f32, bf = dt.float32, dt.bfloat16

    nc = bacc.Bacc("TRN2", target_bir_lowering=False, debug=False,
                   num_devices=num_devices)

    # all inputs host-swizzled to per-partition-contiguous layouts so each
    # DMA is 128 large contiguous runs (fast descriptor generation)
    xT = nc.dram_tensor("xT", [P, KC, NQ], bf, kind="ExternalInput").ap()
    ctxT = nc.dram_tensor("ctxT", [NT, P, KC, 512], bf,
                          kind="ExternalInput").ap()
    wqT = nc.dram_tensor("wqT", [P, KC, INNER], bf, kind="ExternalInput").ap()
    wkT = nc.dram_tensor("wkT", [P, KC, INNER], bf, kind="ExternalInput").ap()
    wvT = nc.dram_tensor("wvT", [P, KC, INNER], bf, kind="ExternalInput").ap()
    woT = nc.dram_tensor("woT", [P, ICK, D], bf, kind="ExternalInput").ap()
    if need_mask:
        maskb = nc.dram_tensor("maskb", [P, NCHUNK], f32,
                               kind="ExternalInput").ap()
    y = nc.dram_tensor("y", [NQ, D], f32, kind="ExternalOutput").ap()

    Exp = mybir.ActivationFunctionType.Exp

    with tile.TileContext(nc) as tc:
        with tc.tile_pool(name="big", bufs=1) as big, \
             tc.tile_pool(name="work", bufs=3) as work, \
             tc.tile_pool(name="pTp", bufs=6) as pTp, \
             tc.tile_pool(name="dram", bufs=2, space="DRAM") as dram, \
             tc.tile_pool(name="proj_ps", bufs=2, space="PSUM") as proj_ps, \
             tc.tile_pool(name="score_ps", bufs=2, space="PSUM") as score_ps, \
             tc.tile_pool(name="av_ps", bufs=2, space="PSUM") as av_ps:

            ctx_sb = big.tile([P, NT, KC, 512], bf, name="ctx_sb")
            x_sb = big.tile([P, KC, NQ], bf, name="x_sb")
            wq_sb = big.tile([P, KC, INNER], bf, name="wq_sb")
            wk_sb = big.tile([P, KC, INNER], bf, name="wk_sb")
            wv_sb = big.tile([P, KC, INNER], bf, name="wv_sb")
            wo_sb = big.tile([P, ICK, D], bf, name="wo_sb")
            q_sb = big.tile([P, ICK, NQ], bf, name="q_sb")
            k_sb = big.tile([P, ICK, NKV], bf, name="k_sb")
            v_sb = big.tile([P, NCHUNK, NHL, DIM_HEAD + 1], bf, name="v_sb")
            o_sb = big.tile([P, ICK, NQ], bf, name="o_sb")
            if need_mask:
                mb_sb = big.tile([P, NCHUNK], f32, name="mb_sb")

            # ---- input DMAs spread over the 3 DMA-capable queues ----
            nc.scalar.dma_start(wq_sb[:], wqT)
            nc.gpsimd.dma_start(wk_sb[:], wkT)
            nc.scalar.dma_start(wv_sb[:], wvT)
            nc.sync.dma_start(x_sb[:], xT)
            ctx_eng = [nc.gpsimd, nc.gpsimd, nc.sync, nc.gpsimd,
                       nc.gpsimd, nc.sync, nc.scalar, nc.scalar]
            for nt in range(NT):
                ctx_eng[nt].dma_start(ctx_sb[:, nt, :, :], ctxT[nt])
            nc.scalar.dma_start(wo_sb[:], woT)
            if need_mask:
                nc.scalar.dma_start(mb_sb[:], maskb[:])
            nc.vector.memset(v_sb[:, :, :, DIM_HEAD], 1.0)

            # ---- Q proj: qT[256, 1024] ----
            for ic in range(ICK):
                for qh in range(NQH):
                    ps = proj_ps.tile([P, 512], f32, name="ps_proj",
                                      tag="proj")
                    for kc in range(KC):
                        nc.tensor.matmul(
                            ps, wq_sb[:, kc, ic * P:(ic + 1) * P],
                            x_sb[:, kc, qh * 512:(qh + 1) * 512],
                            start=(kc == 0), stop=(kc == KC - 1))
                    nc.vector.tensor_copy(
                        out=q_sb[:, ic, qh * 512:(qh + 1) * 512], in_=ps)

            def emit_k(ic, nt):
                ps = proj_ps.tile([P, 512], f32, name="ps_proj", tag="proj")
                for kc in range(KC):
                    nc.tensor.matmul(
                        ps, wk_sb[:, kc, ic * P:(ic + 1) * P],
                        ctx_sb[:, nt, kc, :],
                        start=(kc == 0), stop=(kc == KC - 1))
                nc.vector.tensor_copy(
                    out=k_sb[:, ic, nt * 512:(nt + 1) * 512], in_=ps)

            def emit_v(j):
                # v rows for kv chunk j, all 4 local heads: [128kv, 256]
                nt, jo = j // 4, (j % 4) * P
                ps = proj_ps.tile([P, 512], f32, name="ps_proj", tag="proj")
                for kc in range(KC):
                    nc.tensor.matmul(
                        ps[:, 0:INNER], ctx_sb[:, nt, kc, jo:jo + P],
                        wv_sb[:, kc, :], start=(kc == 0), stop=(kc == KC - 1))
                nc.vector.tensor_copy(
                    out=v_sb[:, j, :, 0:DIM_HEAD],
                    in_=ps[:, 0:INNER].rearrange("p (h d) -> p h d", h=NHL))

            y_eng = [nc.gpsimd, nc.sync, nc.gpsimd, nc.sync,
                     nc.scalar, nc.sync, nc.gpsimd, nc.scalar]

            def emit_oproj(qc):
                # y_partial chunk = O[qc] @ Wo_hg^T
                ps = proj_ps.tile([P, 512], f32, name="ps_proj", tag="proj")
                for ic in range(ICK):
                    nc.tensor.matmul(
                        ps, o_sb[:, ic, qc * P:(qc + 1) * P],
                        wo_sb[:, ic, :], start=(ic == 0), stop=(ic == ICK - 1))
                y_sb = work.tile([P, D], f32, name="y_sb", tag="y_sb")
                nc.vector.tensor_copy(out=y_sb, in_=ps)
                y_eng[qc].dma_start(y[qc * P:(qc + 1) * P, :], y_sb)

            # prologue: k ic0 nt0/nt1 and v chunks 0,1 so segment (0,0)
            # never waits at its head.
            emit_k(0, 0)
            emit_v(0)
            emit_k(0, 1)
            emit_v(1)

            SKEW = 3
            for hp in range(ICK):          # head pair = inner chunk
                h0, h1 = 2 * hp, 2 * hp + 1
                for qh in range(NQH):
                    seg0 = (hp == 0 and qh == 0)
                    seg1 = (hp == 0 and qh == 1)
                    seg2 = (hp == 1 and qh == 0)
                    seg3 = (hp == 1 and qh == 1)
                    av0 = av_ps.tile([DIM_HEAD + 1, 512], f32, name="av0",
                                     tag="av")
                    av1 = av_ps.tile([DIM_HEAD + 1, 512], f32, name="av1",
                                     tag="av")
                    pend = []

                    def emit_av(j, pT, av0=av0, av1=av1, h0=h0, h1=h1):
                        nc.tensor.matmul(
                            av0, v_sb[:, j, h0, :], pT[:, 0:512],
                            start=(j == 0), stop=(j == NCHUNK - 1))
                        nc.tensor.matmul(
                            av1, v_sb[:, j, h1, :], pT[:, 512:1024],
                            start=(j == 0), stop=(j == NCHUNK - 1))

                    for j in range(NCHUNK):
                        if seg0:
                            if j + 2 < NCHUNK:
                                emit_v(j + 2)
                            if j % 4 == 2 and j < 24:
                                emit_k(0, j // 4 + 2)
                        if seg1 and j in (21, 25):
                            emit_k(1, (j - 21) // 4)
                        if seg2 and j % 4 == 0 and j < 24:
                            emit_k(1, j // 4 + 2)
                        if seg3 and j in (6, 8, 10, 12):
                            emit_oproj((j - 6) // 2)

                        sc = score_ps.tile([P, 1024], f32, name="sc",
                                           tag="sc")
                        nc.tensor.matmul(
                            sc[:, 0:512],
                            k_sb[0:DIM_HEAD, hp, j * P:(j + 1) * P],
                            q_sb[0:DIM_HEAD, hp, qh * 512:(qh + 1) * 512],
                            start=True, stop=True)
                        nc.tensor.matmul(
                            sc[:, 512:1024],
                            k_sb[DIM_HEAD:P, hp, j * P:(j + 1) * P],
                            q_sb[DIM_HEAD:P, hp, qh * 512:(qh + 1) * 512],
                            start=True, stop=True)
                        pT = pTp.tile([P, 1024], bf, name="pT", tag="pT")
                        if need_mask:
                            nc.scalar.activation(
                                pT[:], sc[:], Exp,
                                bias=mb_sb[:, j, None], scale=0.125)
                        else:
                            nc.scalar.activation(pT[:], sc[:], Exp,
                                                 scale=0.125)
                        pend.append((j, pT))
                        if len(pend) > SKEW:
                            emit_av(*pend.pop(0))
                    for item in pend:
                        emit_av(*item)

                    # ---- normalize: stage PSUM->SBUF, in-place fast recip
                    # of the sum rows, DMA-broadcast, scale, write O^T ----
                    avs = work.tile([DIM_HEAD + 1, 2, 512], f32, name="avs",
                                    tag="avs")
                    nc.vector.tensor_copy(out=avs[:, 0, :], in_=av0[:])
                    nc.vector.tensor_copy(out=avs[:, 1, :], in_=av1[:])
                    # pack the 1024 z values across 128 partitions so the
                    # reciprocal is partition-parallel (free size 8, ~100ns
                    # instead of 6.5us on one partition)
                    zrow = avs[DIM_HEAD:DIM_HEAD + 1, :, :]
                    zp = work.tile([P, 8], f32, name="zp", tag="zp")
                    nc.gpsimd.dma_start(zp[:], zrow)
                    zr = work.tile([P, 8], f32, name="zr", tag="zr")
                    nc.vector.reciprocal(zr[:], zp[:])
                    rec_dr = dram.tile([2, DIM_HEAD, 8], f32, name="rec_dr",
                                       tag="rec_dr")
                    nc.gpsimd.dma_start(
                        rec_dr.rearrange("a p f -> (a p) f"), zr[:])
                    rec_bc = work.tile([DIM_HEAD, 2, 512], f32, name="rec_bc",
                                       tag="rec_bc")
                    nc.gpsimd.dma_start(
                        rec_bc[:, 0, :],
                        rec_dr[0:1, :, :].rearrange("a p f -> a (p f)")
                        .to_broadcast([DIM_HEAD, 512]))
                    nc.gpsimd.dma_start(
                        rec_bc[:, 1, :],
                        rec_dr[1:2, :, :].rearrange("a p f -> a (p f)")
                        .to_broadcast([DIM_HEAD, 512]))
                    o_tmp = work.tile([DIM_HEAD, 2, 512], bf, name="o_tmp",
                                      tag="o_tmp")
                    nc.vector.tensor_mul(o_tmp[:, 0, :],
                                         avs[0:DIM_HEAD, 0, :],
                                         rec_bc[:, 0, :])
                    nc.vector.tensor_mul(o_tmp[:, 1, :],
                                         avs[0:DIM_HEAD, 1, :],
                                         rec_bc[:, 1, :])
                    nc.sync.dma_start(
                        o_sb[0:DIM_HEAD, hp, qh * 512:(qh + 1) * 512],
                        o_tmp[:, 0, :])
                    nc.gpsimd.dma_start(
                        o_sb[DIM_HEAD:P, hp, qh * 512:(qh + 1) * 512],
                        o_tmp[:, 1, :])

            # ---- y_partial qc 4-7 (qc 0-3 emitted inside seg3) ----
            for qc in range(4, NQ // P):
                emit_oproj(qc)

    nc.compile()
    return nc


def _get_program(need_mask: bool):
    if need_mask not in _PROGRAMS:
        _PROGRAMS[need_mask] = _build(need_mask)
    return _PROGRAMS[need_mask]


def _prep_inputs(x, context, mask, Wq, Wkv, Wo, bo):
    """Host-side shard + transpose + cast. Returns (in_maps, need_mask, bo)."""
    x = np.asarray(x, dtype=np.float32)
    context = np.asarray(context, dtype=np.float32)
    mask = np.asarray(mask)
    Wq = np.asarray(Wq, dtype=np.float32)
    Wkv = np.asarray(Wkv, dtype=np.float32)
    Wo = np.asarray(Wo, dtype=np.float32)
    bo = np.asarray(bo, dtype=np.float32)

    need_mask = not bool(mask.all())

    def swz(aT, ck):
        # [ck*128, N] -> per-partition-contiguous [128, ck, N]
        return np.ascontiguousarray(
            aT.reshape(ck, P, -1).transpose(1, 0, 2)).astype(BF16)

    xTs = [swz(x[b].T, KC) for b in range(B)]
    # ctx: [512, 4096] -> [nt, p, kc, 512]
    ctxTs = [np.ascontiguousarray(
        context[b].T.reshape(KC, P, NT, 512).transpose(2, 1, 0, 3))
        .astype(BF16) for b in range(B)]
    wqTs, wkTs, wvTs, woTs = [], [], [], []
    for hg in range(2):
        sl = slice(hg * INNER, (hg + 1) * INNER)
        wqTs.append(swz(Wq[sl].T, KC))
        wkTs.append(swz(Wkv[:D][sl].T, KC))
        wvTs.append(swz(Wkv[D:][sl].T, KC))
        woTs.append(swz(Wo[:, sl].T, ICK))
    if need_mask:
        # additive pre-exp bias: 0 where visible, -1e30 where masked
        mb = [np.where(mask[b], 0.0, -1e30).astype(np.float32)
              .reshape(NCHUNK, P).T.copy() for b in range(B)]

    in_maps = []
    for c in range(N_CORES):
        b, hg = divmod(c, 2)
        m = {
            "xT": xTs[b], "ctxT": ctxTs[b],
            "wqT": wqTs[hg], "wkT": wkTs[hg], "wvT": wvTs[hg],
            "woT": woTs[hg],
        }
        if need_mask:
            m["maskb"] = mb[b]
        in_maps.append(m)
    return in_maps, need_mask, bo


def run_sharded(inputs, trace=False):
    """Run on 8 cores; returns (full_output, BassKernelResults)."""
    from concourse import bass_utils
    in_maps, need_mask, bo = _prep_inputs(**inputs)
    nc = _get_program(need_mask)
    res = bass_utils.run_bass_kernel_spmd(
        nc, in_maps, core_ids=list(range(N_CORES)), trace=trace)
    out = np.empty((B, NQ, D), dtype=np.float32)
    for b in range(B):
        out[b] = res.results[2 * b]["y"]
        out[b] += res.results[2 * b + 1]["y"]
        out[b] += bo
    return out, res


def kernel(**inputs) -> np.ndarray:
    out, _ = run_sharded(inputs, trace=False)
    return out
